# revision 19
# baseline (speedup 1.0000x reference)
"""AttentionBlock kernel for 8 Trainium2 NeuronCores.

Sharding: one (batch, head) pair per core (B=2 x HEADS=4 = 8 cores).
Each core:
  - loads x[b] [256, 4096] and computes GroupNorm(8 groups) on-chip (fp32)
  - computes q/k/v for its head via 1x1-conv matmuls (fp16 operands,
    fp32 PSUM accumulation; q and its bias pre-scaled by 1/sqrt(hd))
  - computes S^T = K^T.T @ Q^T per 128-pixel j-block, exp on ScalarE
    (no max-subtraction -- scores are O(10) so fp32 exp is safe),
    row-sums via an appended ones-column in the AV matmul, normalizes,
    applies its slice of the projection, and adds (x + b_proj)/4.
Host sums the 4 per-head partial outputs of each batch.
"""

from contextlib import ExitStack

import numpy as np

import concourse.bass as bass
import concourse.tile as tile
from concourse import bacc, mybir
from concourse.bass_utils import run_bass_kernel_spmd
from concourse.masks import make_identity

F32 = mybir.dt.float32
F16 = mybir.dt.float16
AF = mybir.ActivationFunctionType
OP = mybir.AluOpType

C = 256
N = 4096
HD = 64
HEADS = 4
B = 2
EPS = 1e-5
IG = 1024  # i-group width (exp call width)
NG = N // IG  # 4
JB = 128  # j-block
NJ = N // JB  # 32


def build_nc():
    nc = bacc.Bacc("TRN2", target_bir_lowering=False, debug=False, num_devices=1)

    xb = nc.dram_tensor("xb", [C, N], F32, kind="ExternalInput")
    gw = nc.dram_tensor("gw", [C, 1], F32, kind="ExternalInput")
    bw = nc.dram_tensor("bw", [C, 1], F32, kind="ExternalInput")
    ind = nc.dram_tensor("ind", [128, 4], F32, kind="ExternalInput")
    indT = nc.dram_tensor("indT", [4, 128], F32, kind="ExternalInput")
    wq = nc.dram_tensor("wq", [C, HD], F16, kind="ExternalInput")
    wk = nc.dram_tensor("wk", [C, HD], F16, kind="ExternalInput")
    wv = nc.dram_tensor("wv", [C, HD], F16, kind="ExternalInput")
    bqkv = nc.dram_tensor("bqkv", [HD, 3], F32, kind="ExternalInput")
    wp = nc.dram_tensor("wp", [HD + 1, C], F16, kind="ExternalInput")
    sel = nc.dram_tensor("sel", [HD + 1, HD + 1], F16, kind="ExternalInput")
    out = nc.dram_tensor("out", [C, N], F32, kind="ExternalOutput")

    with tile.TileContext(nc) as tc, ExitStack() as ctx:
        big = ctx.enter_context(tc.tile_pool(name="big", bufs=32))
        xpool = ctx.enter_context(tc.tile_pool(name="xpool", bufs=8))
        qkp = ctx.enter_context(tc.tile_pool(name="qkp", bufs=1))
        cst = ctx.enter_context(tc.tile_pool(name="cst", bufs=1))
        sml = ctx.enter_context(tc.tile_pool(name="sml", bufs=2))
        otnp = ctx.enter_context(tc.tile_pool(name="otnp", bufs=3))
        xrp = ctx.enter_context(tc.tile_pool(name="xrp", bufs=4))
        outp = ctx.enter_context(tc.tile_pool(name="outp", bufs=4))
        rwp = ctx.enter_context(tc.tile_pool(name="rwp", bufs=2))
        ps_s = ctx.enter_context(tc.tile_pool(name="ps_s", bufs=2, space="PSUM"))
        ps_av = ctx.enter_context(tc.tile_pool(name="ps_av", bufs=1, space="PSUM"))
        ps_m = ctx.enter_context(tc.tile_pool(name="ps_m", bufs=3, space="PSUM"))

        # ---- constants ----
        ind_sb = cst.tile([128, 4], F32, tag="ind")
        nc.sync.dma_start(out=ind_sb, in_=ind[:, :])
        indT_sb = cst.tile([4, 128], F32, tag="indT")
        nc.sync.dma_start(out=indT_sb, in_=indT[:, :])
        gw_sb = cst.tile([128, 2, 1], F32, tag="gw")
        nc.sync.dma_start(out=gw_sb, in_=gw[:, :].rearrange("(c p) o -> p c o", p=128))
        bw_sb = cst.tile([128, 2, 1], F32, tag="bw")
        nc.sync.dma_start(out=bw_sb, in_=bw[:, :].rearrange("(c p) o -> p c o", p=128))
        wq_sb = cst.tile([128, 2, HD], F16, tag="wq")
        nc.sync.dma_start(out=wq_sb, in_=wq[:, :].rearrange("(c p) d -> p c d", p=128))
        wk_sb = cst.tile([128, 2, HD], F16, tag="wk")
        nc.sync.dma_start(out=wk_sb, in_=wk[:, :].rearrange("(c p) d -> p c d", p=128))
        wv_sb = cst.tile([128, 2, HD], F16, tag="wv")
        nc.sync.dma_start(out=wv_sb, in_=wv[:, :].rearrange("(c p) d -> p c d", p=128))
        bqkv_sb = cst.tile([HD, 3], F32, tag="bqkv")
        nc.sync.dma_start(out=bqkv_sb, in_=bqkv[:, :])
        wp_sb = cst.tile([HD + 1, 2, 128], F16, tag="wp")
        nc.sync.dma_start(out=wp_sb, in_=wp[:, :].rearrange("a (c m) -> a c m", m=128))
        sel_sb = cst.tile([HD + 1, HD + 1], F16, tag="sel")
        nc.sync.dma_start(out=sel_sb, in_=sel[:, :])
        onescol = cst.tile([128, 1], F16, tag="onescol")
        nc.vector.memset(onescol, 1.0)
        onesrow = cst.tile([1, 512], F16, tag="onesrow")
        nc.vector.memset(onesrow, 1.0)
        iden = cst.tile([HD, HD], F16, tag="iden")
        make_identity(nc, iden)

        # ---- load x: 2 partition-halves x 4 chunks of 1024 ----
        xt = []
        for t in range(2):
            row = []
            for c in range(4):
                tl = xpool.tile([128, 1024], F32, tag="x")
                nc.sync.dma_start(
                    out=tl, in_=xb[128 * t : 128 * (t + 1), 1024 * c : 1024 * (c + 1)]
                )
                row.append(tl)
            xt.append(row)

        # ---- group norm (fp32) ----
        scale_h, shift_h = [], []
        for t in range(2):
            stats = sml.tile([128, 8, 6], F32, tag="stats")
            for c in range(4):
                for s in range(2):
                    nc.vector.bn_stats(
                        out=stats[:, c * 2 + s, :],
                        in_=xt[t][c][:, 512 * s : 512 * (s + 1)],
                    )
            mv = sml.tile([128, 2], F32, tag="mv")
            nc.vector.bn_aggr(out=mv, in_=stats)
            # sm = [mean_c, mean_c^2 + var_c]
            sm = sml.tile([128, 2], F32, tag="sm")
            nc.vector.tensor_copy(sm[:, 0:1], mv[:, 0:1])
            nc.vector.tensor_tensor(sm[:, 1:2], mv[:, 0:1], mv[:, 0:1], OP.mult)
            nc.vector.tensor_tensor(sm[:, 1:2], sm[:, 1:2], mv[:, 1:2], OP.add)
            gs_ps = ps_m.tile([4, 2], F32, tag="m")
            nc.tensor.matmul(gs_ps, lhsT=ind_sb, rhs=sm, start=True, stop=True)
            gstat = sml.tile([4, 2], F32, tag="gstat")  # [gmean, E[x^2]_g]
            nc.vector.tensor_scalar_mul(gstat, gs_ps, 1.0 / 32.0)
            gve = sml.tile([4, 1], F32, tag="gve")  # var_g + eps
            nc.vector.tensor_tensor(gve, gstat[:, 0:1], gstat[:, 0:1], OP.mult)
            nc.vector.scalar_tensor_tensor(
                gve, gve, -1.0, gstat[:, 1:2], OP.mult, OP.add
            )
            nc.vector.tensor_scalar_add(gve, gve, EPS)
            sq = sml.tile([4, 1], F32, tag="sq")
            nc.scalar.activation(sq, gve, AF.Sqrt)
            y0 = sml.tile([4, 1], F32, tag="y0")
            nc.vector.reciprocal(y0, sq)
            # one Newton step: y = y0 * (1.5 - 0.5 * gve * y0^2)
            t1 = sml.tile([4, 1], F32, tag="t1")
            nc.vector.tensor_tensor(t1, y0, y0, OP.mult)
            nc.vector.tensor_tensor(t1, t1, gve, OP.mult)
            nc.vector.tensor_scalar(t1, t1, -0.5, 1.5, op0=OP.mult, op1=OP.add)
            nc.vector.tensor_tensor(y0, y0, t1, OP.mult)
            gfin = sml.tile([4, 2], F32, tag="gfin")  # [gmean, rstd]
            nc.vector.tensor_copy(gfin[:, 0:1], gstat[:, 0:1])
            nc.vector.tensor_copy(gfin[:, 1:2], y0)
            bc_ps = ps_m.tile([128, 2], F32, tag="m")
            nc.tensor.matmul(bc_ps, lhsT=indT_sb, rhs=gfin, start=True, stop=True)
            sc = sml.tile([128, 1], F32, tag=f"sc{t}")
            sh = sml.tile([128, 1], F32, tag=f"sh{t}")
            nc.vector.tensor_tensor(sc, bc_ps[:, 1:2], gw_sb[:, t, :], OP.mult)
            nc.vector.tensor_tensor(sh, bc_ps[:, 0:1], sc, OP.mult)
            nc.vector.tensor_tensor(sh, bw_sb[:, t, :], sh, OP.subtract)
            scale_h.append(sc)
            shift_h.append(sh)

        # ---- normalize: xn = x * scale_c + shift_c (cast to fp16) ----
        xn = []
        for t in range(2):
            row = []
            for c in range(4):
                tl = big.tile([128, 1024], F16, tag="slab16")
                nc.vector.tensor_scalar(
                    tl, xt[t][c], scale_h[t], shift_h[t], op0=OP.mult, op1=OP.add
                )
                row.append(tl)
            xn.append(row)

        # ---- qkv (channel-major, fp16): qT/kT [64, 4096], v chunks [64, 1024] ----
        qT = qkp.tile([128, N], F16, tag="qT")
        kT = qkp.tile([128, N], F16, tag="kT")
        vc = []
        for _ in range(4):
            vtile = big.tile([HD, 1024], F16, tag="slab16")
            vc.append(vtile)
        for ih in range(8):  # 512-wide i-chunks
            tidx, sl = ih // 2, (ih % 2) * 512
            for wi, w_sb in enumerate((wq_sb, wk_sb, wv_sb)):
                ps = ps_m.tile([HD, 512], F32, tag="m")
                for ci in range(2):
                    nc.tensor.matmul(
                        ps,
                        lhsT=w_sb[:, ci, :],
                        rhs=xn[ci][tidx][:, sl : sl + 512],
                        start=(ci == 0),
                        stop=(ci == 1),
                    )
                if wi == 0:
                    dst = qT[0:HD, ih * 512 : (ih + 1) * 512]
                elif wi == 1:
                    dst = kT[0:HD, ih * 512 : (ih + 1) * 512]
                else:
                    dst = vc[tidx][:, sl : sl + 512]
                nc.vector.tensor_scalar_add(dst, ps, bqkv_sb[:, wi : wi + 1])

        # duplicate q/k onto partitions 64:127 for row-packed S^T matmuls
        nc.sync.dma_start(out=qT[HD:128, :], in_=qT[0:HD, :])
        nc.sync.dma_start(out=kT[HD:128, :], in_=kT[0:HD, :])

        # ---- transpose v into vones [128, 32, 64+1] (col HD = ones) ----
        vones = cst.tile([128, NJ, HD + 1], F16, tag="vones")
        for j in range(NJ):
            nc.vector.tensor_copy(vones[:, j, HD : HD + 1], onescol)
            tp = ps_m.tile([128, HD], F16, tag="m")
            nc.tensor.transpose(
                tp, vc[j // 8][:, (j % 8) * 128 : (j % 8 + 1) * 128], iden
            )
            nc.vector.tensor_copy(vones[:, j, 0:HD], tp)

        # ---- attention + projection, per 512-wide i-chunk ----
        # S^T is row-packed: j-pair (2*jp, 2*jp+1) runs as two concurrent
        # K=64 matmuls in PE row groups 0-63 / 64-127, writing the two
        # banks of one [128, 1024] PSUM tile. The exp slab tile holds
        # [j0 | j1] x 512 i side by side. The softmax divide and the
        # projection are software-pipelined one/two i-chunks behind the
        # S^T stream so the slow DVE divide stays off the PE critical
        # path.
        slabs_by_ic = {}
        av_by_ic = {}
        sm_by_ic = {}

        def st_pack(icx, jp):
            i0 = icx * 512
            sps = ps_s.tile([128, 1024], F32, tag="s")
            nc.tensor.matmul(
                sps[:, 0:512],
                lhsT=kT[0:HD, JB * 2 * jp : JB * (2 * jp + 1)],
                rhs=qT[0:HD, i0 : i0 + 512],
                start=True,
                stop=True,
            )
            nc.tensor.matmul(
                sps[:, 512:1024],
                lhsT=kT[HD:128, JB * (2 * jp + 1) : JB * (2 * jp + 2)],
                rhs=qT[HD:128, i0 : i0 + 512],
                start=True,
                stop=True,
            )
            st = big.tile([128, 1024], F16, tag="slab16")
            nc.scalar.activation(st, sps, AF.Exp)
            slabs_by_ic.setdefault(icx, []).append(st)

        def av_pair(icx, jp):
            if jp == 0:
                av_t = ps_av.tile([HD + 1, 512], F32, tag="av")
                av_by_ic[icx] = av_t
            av = av_by_ic[icx]
            slabs = slabs_by_ic[icx]
            for j in (2 * jp, 2 * jp + 1):
                nc.tensor.matmul(
                    av,
                    lhsT=vones[:, j, :],
                    rhs=slabs[j // 2][:, 512 * (j % 2) : 512 * (j % 2 + 1)],
                    start=(j == 0),
                    stop=(j == NJ - 1),
                )

        def stage_sm(icx):
            del slabs_by_ic[icx]
            av = av_by_ic.pop(icx)
            ocp = rwp.tile([HD + 1, 512], F16, tag="ocp")
            nc.vector.tensor_copy(ocp, av)
            bc = ps_m.tile([HD + 1, 512], F32, tag="m")
            nc.tensor.matmul(bc, lhsT=sel_sb, rhs=ocp, start=True, stop=True)
            rbc = rwp.tile([HD + 1, 512], F32, tag="rbc")
            nc.vector.reciprocal(rbc, bc)
            otn = otnp.tile([HD + 1, 512], F16, tag="otn")
            nc.vector.tensor_copy(otn[HD : HD + 1, :], onesrow)
            nc.vector.tensor_tensor(
                otn[0:HD, :], ocp[0:HD, :], rbc[0:HD, :], OP.mult
            )
            sm_by_ic[icx] = otn

        def stage_pj(icx):
            otn = sm_by_ic.pop(icx)
            i0 = icx * 512
            for m in range(2):
                pp = ps_m.tile([128, 512], F32, tag="m")
                nc.tensor.matmul(
                    pp, lhsT=wp_sb[:, m, :], rhs=otn, start=True, stop=True
                )
                xr = xrp.tile([128, 512], F32, tag="xr")
                nc.sync.dma_start(
                    out=xr, in_=xb[128 * m : 128 * (m + 1), i0 : i0 + 512]
                )
                ot = outp.tile([128, 512], F32, tag="ot")
                nc.vector.scalar_tensor_tensor(ot, xr, 0.25, pp, OP.mult, OP.add)
                nc.sync.dma_start(
                    out=out[128 * m : 128 * (m + 1), i0 : i0 + 512], in_=ot
                )

        NP = NJ // 2  # 16 packs per i-chunk
        for icx in range(8):
            for jp in range(NP):
                st_pack(icx, jp)
                if jp >= 2:
                    av_pair(icx, jp - 2)
                if jp == 6 and icx >= 1:
                    stage_pj(icx - 1)
            av_pair(icx, NP - 2)
            av_pair(icx, NP - 1)
            stage_sm(icx)
        stage_pj(7)
    nc.compile()
    return nc


_NC_CACHE = None
_LAST_IN_MAPS = None


def kernel(x, gamma, beta, w_qkv, b_qkv, w_proj, b_proj):
    global _NC_CACHE, _LAST_IN_MAPS
    if _NC_CACHE is None:
        _NC_CACHE = build_nc()
    nc = _NC_CACHE

    x = np.asarray(x, dtype=np.float32)
    gamma = np.asarray(gamma, dtype=np.float32)
    beta = np.asarray(beta, dtype=np.float32)
    w_qkv = np.asarray(w_qkv, dtype=np.float32)
    b_qkv = np.asarray(b_qkv, dtype=np.float32)
    w_proj = np.asarray(w_proj, dtype=np.float32)
    b_proj = np.asarray(b_proj, dtype=np.float32)

    scale = float(HD) ** -0.5
    ind = np.zeros((128, 4), np.float32)
    for cc in range(128):
        ind[cc, cc // 32] = 1.0
    indT = np.ascontiguousarray(ind.T)
    sel_np = np.zeros((HD + 1, HD + 1), np.float16)
    sel_np[HD, :] = 1.0

    in_maps = []
    for core in range(8):
        b, h = divmod(core, HEADS)
        qr = slice(HD * h, HD * (h + 1))
        kr = slice(C + HD * h, C + HD * (h + 1))
        vr = slice(2 * C + HD * h, 2 * C + HD * (h + 1))
        bq = b_qkv[qr] * scale
        bk = b_qkv[kr]
        bv = b_qkv[vr]
        in_maps.append(
            {
                "xb": np.ascontiguousarray(x[b].reshape(C, N)),
                "gw": np.ascontiguousarray(gamma.reshape(C, 1)),
                "bw": np.ascontiguousarray(beta.reshape(C, 1)),
                "ind": ind,
                "indT": indT,
                "wq": np.ascontiguousarray((w_qkv[qr].T * scale).astype(np.float16)),
                "wk": np.ascontiguousarray(w_qkv[kr].T.astype(np.float16)),
                "wv": np.ascontiguousarray(w_qkv[vr].T.astype(np.float16)),
                "bqkv": np.ascontiguousarray(np.stack([bq, bk, bv], axis=1)),
                "wp": np.ascontiguousarray(
                    np.concatenate(
                        [w_proj[:, HD * h : HD * (h + 1)].T, (b_proj * 0.25)[None, :]],
                        axis=0,
                    ).astype(np.float16)
                ),
                "sel": sel_np,
            }
        )

    _LAST_IN_MAPS = in_maps
    res = run_bass_kernel_spmd(nc, in_maps, core_ids=list(range(8)))
    full = np.zeros((B, C, N), np.float32)
    for core in range(8):
        full[core // HEADS] += res.results[core]["out"]
    return full.reshape(B, C, 64, 64)
